# revision 27
# baseline (speedup 1.0000x reference)
"""MLA (multi-head latent attention) forward on 8 Trainium2 NeuronCores.

Sharding: tensor-parallel over heads (4 groups of 4 heads) x data-parallel
over batch (2), giving 8 cores. wq / wkv_b rows and wo columns are sharded by
head; each core computes the full latent kv_c / k_pe for its batch element
locally (replicated within the 4-core head group -- cheaper than an
AllGather, whose latency head-of-line-blocks the PE queue). Each core
produces a partial [S, DIM] output (its heads' contribution through wo); the
host sums the 4 head-group partials per batch element.

Per-core dataflow is fully "transposed" (feature dims on SBUF partitions,
sequence on the free dim):
  per 512-seq chunk: kv = x @ wkv_a^T (RMS-normed, k_pe roped),
    qT = wq_g @ x^T (nope rows, and rope rows split into lo/hi pairs),
    kv_cT / k_peT via PE transposes at chunk end
  attention per (chunk, head): absorbed qprojT = wbk^T q, then per 128-t-tile
    scoresT[t, sq]; exp (no max subtraction -- scores are O(1)); causal
    masking by 0/1 multiply on diagonal tiles
  oT[c, sq] += kv_c[t-tile].T @ exp_scoresT ; l[sq] via ones-row matmul
  o_final = wb_v^T @ oT, normalized by broadcast(1/l) (PE K=1 matmul trick)
  out[s, :] = o_catT^T @ woT (partial over this core's heads)
"""

import numpy as np
import ml_dtypes

import concourse.bass as bass
import concourse.tile as tile
import concourse.mybir as mybir
from concourse import bass_utils

BF16 = mybir.dt.bfloat16
F32 = mybir.dt.float32
F32R = mybir.dt.float32r
AF = mybir.ActivationFunctionType
ALU = mybir.AluOpType
NPBF16 = ml_dtypes.bfloat16

import os

B, S, DIM, H = 2, 2048, 2048, 16
_QCUT = int(os.environ.get("MLA_DEBUG_QCUT", "4"))  # debug: attention chunks
NOPE, ROPE, VHD, KLR = 128, 64, 128, 512
QKHD = NOPE + ROPE
SCALE = QKHD ** -0.5
EPS = 1.1920929e-07
P = 128
HG = 4            # heads per core
CH = 512          # sequence chunk (matmul free dim)
NCH = S // CH     # 4 chunks
NT = S // P       # 16 tiles of 128
KH = 8            # k-tiles per x half-load


def _emit(nc, reps=1):
    dt = nc.dram_tensor
    xT = dt("xT", [P, NCH, NT, CH], BF16, kind="ExternalInput").ap()
    wqn = dt("wqn", [P, NT, 512], BF16, kind="ExternalInput").ap()
    wqlo = dt("wqlo", [P, NT, 128], BF16, kind="ExternalInput").ap()
    wqhi = dt("wqhi", [P, NT, 128], BF16, kind="ExternalInput").ap()
    wkva = dt("wkva", [P, NT, 576], BF16, kind="ExternalInput").ap()
    wbk = dt("wbk", [P, HG, KLR], BF16, kind="ExternalInput").ap()
    wbvT = dt("wbvT", [P, 4, HG, VHD], BF16, kind="ExternalInput").ap()
    woT = dt("woT", [P, 4, DIM], BF16, kind="ExternalInput").ap()
    cosS = dt("cosS", [P, S], BF16, kind="ExternalInput").ap()
    sinS = dt("sinS", [P, S], BF16, kind="ExternalInput").ap()
    cosT = dt("cosT", [P, NT, 32], BF16, kind="ExternalInput").ap()
    sinT = dt("sinT", [P, NT, 32], BF16, kind="ExternalInput").ap()
    mmst = dt("mmst", [P, 512], BF16, kind="ExternalInput").ap()
    onec = dt("onec", [P, 1], BF16, kind="ExternalInput").ap()
    oner = dt("oner", [1, P], F32R, kind="ExternalInput").ap()
    ident = dt("ident", [P, P], BF16, kind="ExternalInput").ap()
    epsb = dt("epsb", [P, 1], F32, kind="ExternalInput").ap()
    outp = dt("outp", [P, NT, DIM], F32, kind="ExternalOutput").ap()

    with tile.TileContext(nc) as tc:
        from contextlib import ExitStack

        with ExitStack() as ctx:
            ec = ctx.enter_context
            const = ec(tc.tile_pool(name="const", bufs=1))
            w16 = ec(tc.tile_pool(name="w16", bufs=1))  # wqn then woT
            xpool = ec(tc.tile_pool(name="xpool", bufs=2))
            qpp = ec(tc.tile_pool(name="qpp", bufs=3))
            ocp = ec(tc.tile_pool(name="ocp", bufs=2))
            expp = ec(tc.tile_pool(name="expp", bufs=3))
            ovp = ec(tc.tile_pool(name="ovp", bufs=4))
            stgp = ec(tc.tile_pool(name="stgp", bufs=2))
            f32p = ec(tc.tile_pool(name="f32p", bufs=3))
            smallp = ec(tc.tile_pool(name="smallp", bufs=6))
            statp = ec(tc.tile_pool(name="statp", bufs=4))

            psA = ec(tc.tile_pool(name="psA", bufs=4, space="PSUM"))
            psB = ec(tc.tile_pool(name="psB", bufs=2, space="PSUM"))
            psL = ec(tc.tile_pool(name="psL", bufs=1, space="PSUM"))
            psC = ec(tc.tile_pool(name="psC", bufs=1, space="PSUM"))

            for _rep in range(reps):
                _emit_body(
                    nc, const, w16, xpool, qpp, ocp, expp, ovp, stgp, f32p,
                    smallp, statp, psA, psB, psL, psC,
                    xT, wqn, wqlo, wqhi, wkva, wbk, wbvT, woT, cosS, sinS,
                    cosT, sinT, mmst, onec, oner, ident, epsb, outp,
                )
    return nc


def _emit_body(
    nc, const, w16, xpool, qpp, ocp, expp, ovp, stgp, f32p,
    smallp, statp, psA, psB, psL, psC,
    xT, wqn, wqlo, wqhi, wkva, wbk, wbvT, woT, cosS, sinS,
    cosT, sinT, mmst, onec, oner, ident, epsb, outp,
):
    if True:
        if True:
            # ---- resident weights/tables ----
            # kv-path + chunk-0 data first, split into k-groups so the first
            # matmuls unblock early. x on the sync(SP) queue, weights on the
            # scalar(Act) queue so issue overhead parallelizes.
            x0a = xpool.tile([P, KH, CH], BF16, tag="x")
            x0b = xpool.tile([P, KH, CH], BF16, tag="x")
            wkva_sb = const.tile([P, NT, 576], BF16, tag="wkva")
            wqn_sb = w16.tile([P, NT, 512], BF16, tag="w16", name="wqn_sb")
            for g in range(4):
                ksl = slice(g * 4, (g + 1) * 4)
                x_dst = (x0a, x0b)[g // 2]
                kdst = slice((g % 2) * 4, (g % 2) * 4 + 4)
                nc.sync.dma_start(x_dst[:, kdst, :], xT[:, 0, ksl, :])
                nc.scalar.dma_start(wkva_sb[:, ksl, :], wkva[:, ksl, :])
                nc.scalar.dma_start(wqn_sb[:, ksl, :], wqn[:, ksl, :])
            wqlo_sb = const.tile([P, NT, 128], BF16, tag="wqlo")
            nc.scalar.dma_start(wqlo_sb[:], wqlo)
            wqhi_sb = const.tile([P, NT, 128], BF16, tag="wqhi")
            nc.scalar.dma_start(wqhi_sb[:], wqhi)
            cosS_sb = const.tile([P, S], BF16, tag="cosS")
            nc.scalar.dma_start(cosS_sb[:], cosS)
            sinS_sb = const.tile([P, S], BF16, tag="sinS")
            nc.scalar.dma_start(sinS_sb[:], sinS)
            cosT_sb = const.tile([P, NT, 32], BF16, tag="cosT")
            nc.gpsimd.dma_start(cosT_sb[:], cosT)
            sinT_sb = const.tile([P, NT, 32], BF16, tag="sinT")
            nc.gpsimd.dma_start(sinT_sb[:], sinT)
            mmst_sb = const.tile([P, 512], BF16, tag="mmst")
            nc.gpsimd.dma_start(mmst_sb[:], mmst)
            onec_sb = const.tile([P, 1], BF16, tag="onec")
            nc.gpsimd.dma_start(onec_sb[:], onec)
            oner_sb = const.tile([1, P], F32R, tag="oner")
            nc.gpsimd.dma_start(oner_sb[:], oner)
            ident_sb = const.tile([P, P], BF16, tag="ident")
            nc.gpsimd.dma_start(ident_sb[:], ident)
            eps_sb = const.tile([P, 1], F32, tag="epsb")
            nc.gpsimd.dma_start(eps_sb[:], epsb)
            wbk_sb = const.tile([P, HG, KLR], BF16, tag="wbk")
            nc.scalar.dma_start(wbk_sb[:], wbk)
            wbvT_sb = const.tile([P, 4, HG, VHD], BF16, tag="wbvT")
            nc.scalar.dma_start(wbvT_sb[:], wbvT)

            # ---- persistent activations ----
            qnope_sb = const.tile([P, HG, S], BF16, tag="qnope")
            # head h rope operand: partitions (h%2)*64 + [0,64), index h//2
            qpe_sb = const.tile([P, 2, S], BF16, tag="qpe")
            kvc_sb = const.tile([P, NT, KLR], BF16, tag="kvc")
            kvcT_sb = const.tile([P, 4, S], BF16, tag="kvcT")
            kpeT_sb = const.tile([P, S], BF16, tag="kpeT")  # dup rows 64:128
            krop_sb = const.tile([P, NT, 64], BF16, tag="krop")

            # ================= phase 1: kv + q projections, per chunk ======
            xh = [x0a, x0b]
            for q in range(NCH):
                qs = slice(q * CH, (q + 1) * CH)
                if q > 0:
                    xh = [xpool.tile([P, KH, CH], BF16, tag="x", name="xha"),
                          xpool.tile([P, KH, CH], BF16, tag="x", name="xhb")]
                    nc.sync.dma_start(xh[0][:], xT[:, q, 0:KH, :])
                    nc.sync.dma_start(xh[1][:], xT[:, q, KH:NT, :])

                # kv projection for this chunk's 4 s-tiles, k-outer
                ps_c = [
                    psA.tile([P, KLR], F32, tag="oacc", name=f"kv{j}")
                    for j in range(4)
                ]
                ps_r = psL.tile([P, 4, 64], F32, tag="lacc")
                for k in range(NT):
                    xk = xh[k // KH][:, k % KH, :]
                    for j in range(4):
                        jsl = slice(j * 128, (j + 1) * 128)
                        nc.tensor.matmul(
                            ps_c[j],
                            lhsT=xk[:, jsl],
                            rhs=wkva_sb[:, k, 0:512],
                            start=(k == 0),
                            stop=(k == NT - 1),
                        )
                    for j in range(4):
                        jsl = slice(j * 128, (j + 1) * 128)
                        # one bank holds all 4 chains: only the first chain
                        # may issue start=True (start pending-zeroes the
                        # whole 2KB bank; later chains zero-fill on first
                        # accumulate into pending bytes)
                        nc.tensor.matmul(
                            ps_r[:, j, :],
                            lhsT=xk[:, jsl],
                            rhs=wkva_sb[:, k, 512:576],
                            start=(k == 0 and j == 0),
                            stop=(k == NT - 1),
                            skip_group_check=True,
                        )
                # RMS norm over c; normed kv -> kvc_sb (bf16).
                # rsqrt via exp(-0.5*ln(mean+eps)): ln/exp/square/copy share
                # one activation table, so the Act engine never reloads its
                # function table (Sqrt would force a ~1.3us reload each way).
                for j in range(4):
                    st = q * 4 + j
                    scr = f32p.tile([P, KLR], F32, tag="f32")
                    ssq = statp.tile([P, 1], F32, tag="st")
                    nc.scalar.activation(scr[:], ps_c[j], AF.Square, accum_out=ssq[:])
                    lnv = statp.tile([P, 1], F32, tag="st")
                    nc.scalar.activation(
                        lnv[:], ssq[:], AF.Ln, bias=eps_sb[:], scale=1.0 / KLR
                    )
                    rin = statp.tile([P, 1], F32, tag="st")
                    nc.scalar.activation(rin[:], lnv[:], AF.Exp, scale=-0.5)
                    nc.vector.tensor_scalar_mul(kvc_sb[:, st, :], ps_c[j], rin[:])
                # k_pe rope for the chunk ([t, r] layout)
                kper = smallp.tile([P, 4, 64], BF16, tag="sm")
                nc.scalar.copy(kper[:], ps_r)
                csl = slice(q * 4, (q + 1) * 4)
                u1 = smallp.tile([P, 4, 32], BF16, tag="sm4")
                u2 = smallp.tile([P, 4, 32], BF16, tag="sm4")
                nc.vector.tensor_tensor(
                    u1[:], kper[:, :, 0:32], cosT_sb[:, csl, :], ALU.mult)
                nc.vector.tensor_tensor(
                    u2[:], kper[:, :, 32:64], sinT_sb[:, csl, :], ALU.mult)
                nc.vector.tensor_tensor(
                    krop_sb[:, csl, 0:32], u1[:], u2[:], ALU.subtract)
                u3 = smallp.tile([P, 4, 32], BF16, tag="sm4")
                u4 = smallp.tile([P, 4, 32], BF16, tag="sm4")
                nc.vector.tensor_tensor(
                    u3[:], kper[:, :, 0:32], sinT_sb[:, csl, :], ALU.mult)
                nc.vector.tensor_tensor(
                    u4[:], kper[:, :, 32:64], cosT_sb[:, csl, :], ALU.mult)
                nc.vector.tensor_tensor(
                    krop_sb[:, csl, 32:64], u3[:], u4[:], ALU.add)

                # q projections, k-outer: 6 m-tiles accumulate in parallel
                lo_t = smallp.tile([P, CH], BF16, tag="sm")
                hi_t = smallp.tile([P, CH], BF16, tag="sm")
                qps = [
                    psB.tile([P, CH], F32, tag="mm", name=f"qt{m}") for m in range(2)
                ] + [
                    psA.tile([P, CH], F32, tag="oacc", name=f"qt{m + 2}")
                    for m in range(4)
                ]
                for k in range(NT):
                    xk = xh[k // KH][:, k % KH, :]
                    for m in range(HG):
                        nc.tensor.matmul(
                            qps[m + 2],
                            lhsT=wqn_sb[:, k, m * 128 : (m + 1) * 128],
                            rhs=xk,
                            start=(k == 0),
                            stop=(k == NT - 1),
                        )
                    nc.tensor.matmul(
                        qps[0],
                        lhsT=wqlo_sb[:, k, :],
                        rhs=xk,
                        start=(k == 0),
                        stop=(k == NT - 1),
                    )
                    nc.tensor.matmul(
                        qps[1],
                        lhsT=wqhi_sb[:, k, :],
                        rhs=xk,
                        start=(k == 0),
                        stop=(k == NT - 1),
                    )
                for m in range(HG):
                    if m % 2 == 0:
                        nc.vector.tensor_copy(qnope_sb[:, m, qs], qps[m + 2])
                    else:
                        nc.scalar.copy(qnope_sb[:, m, qs], qps[m + 2])
                nc.vector.tensor_copy(lo_t[:], qps[0])
                nc.scalar.copy(hi_t[:], qps[1])
                # rope on full-width tiles
                t1 = smallp.tile([P, CH], BF16, tag="sm")
                t2 = smallp.tile([P, CH], BF16, tag="sm")
                nc.vector.tensor_tensor(t1[:], lo_t[:], cosS_sb[:, qs], ALU.mult)
                nc.vector.tensor_tensor(t2[:], hi_t[:], sinS_sb[:, qs], ALU.mult)
                nc.vector.tensor_tensor(t1[:], t1[:], t2[:], ALU.subtract)
                t3 = smallp.tile([P, CH], BF16, tag="sm")
                t4 = smallp.tile([P, CH], BF16, tag="sm")
                nc.vector.tensor_tensor(t3[:], lo_t[:], sinS_sb[:, qs], ALU.mult)
                nc.vector.tensor_tensor(t4[:], hi_t[:], cosS_sb[:, qs], ALU.mult)
                nc.vector.tensor_tensor(t3[:], t3[:], t4[:], ALU.add)
                # repack to per-head contiguous [lo;hi] via SBUF->SBUF DMA
                for hh in range(HG):
                    base = (hh % 2) * 64
                    j = hh // 2
                    nc.sync.dma_start(
                        qpe_sb[base : base + 32, j, qs], t1[hh * 32 : (hh + 1) * 32, :]
                    )
                    nc.sync.dma_start(
                        qpe_sb[base + 32 : base + 64, j, qs],
                        t3[hh * 32 : (hh + 1) * 32, :],
                    )

                # transposes for this chunk's kv tiles (chunk end: no HOL)
                for j in range(4):
                    st = q * 4 + j
                    tsl = slice(st * 128, (st + 1) * 128)
                    for cs in range(4):
                        tp = psC.tile([P, P], BF16, tag="aux")
                        nc.tensor.transpose(
                            tp, kvc_sb[:, st, cs * 128 : (cs + 1) * 128], ident_sb[:]
                        )
                        if cs % 2 == 0:
                            nc.vector.tensor_copy(kvcT_sb[:, cs, tsl], tp)
                        else:
                            nc.scalar.copy(kvcT_sb[:, cs, tsl], tp)
                    kp = psC.tile([64, P], BF16, tag="aux")
                    nc.tensor.transpose(kp, krop_sb[:, st, :], ident_sb[:])
                    nc.vector.tensor_copy(kpeT_sb[0:64, tsl], kp)
            # duplicate k_peT into partitions 64:128 (for heads at base 64)
            nc.sync.dma_start(kpeT_sb[64:128, :], kpeT_sb[0:64, :])

            # wo loads late, reusing wqn's slot (dead after phase 1)
            woT_sb = w16.tile([P, 4, DIM], BF16, tag="w16", name="woT_sb")
            nc.scalar.dma_start(woT_sb[:], woT)

            # ================= phase 2: attention =================
            def absorb(qc, hh):
                """Absorbed query projection for (head, chunk) -> qp tile."""
                qs = slice(qc * CH, (qc + 1) * CH)
                qp = qpp.tile([P, 4, CH], BF16, tag="qp")
                for cs in range(4):
                    ps = psB.tile([P, CH], F32, tag="mm")
                    nc.tensor.matmul(
                        ps,
                        lhsT=wbk_sb[:, hh, cs * 128 : (cs + 1) * 128],
                        rhs=qnope_sb[:, hh, qs],
                        start=True,
                        stop=True,
                    )
                    if cs % 2 == 0:
                        nc.vector.tensor_copy(qp[:, cs, :], ps)
                    else:
                        nc.scalar.copy(qp[:, cs, :], ps)
                return qp

            qp_next = absorb(0, 0)
            for qc in range(_QCUT):
                qs = slice(qc * CH, (qc + 1) * CH)
                ocat = ocp.tile([P, HG, CH], BF16, tag="oc")
                for hh in range(HG):
                    qp = qp_next
                    base = (hh % 2) * 64
                    jj = hh // 2
                    q_pe = qpe_sb[base : base + 64, jj, qs]
                    k_pe = kpeT_sb[base : base + 64, :]

                    oacc = [
                        psA.tile([P, CH], F32, tag="oacc", name=f"oacc{i}")
                        for i in range(4)
                    ]
                    l_ps = psL.tile([1, CH], F32, tag="lacc")
                    nti = 4 * qc + 4

                    def emit_oacc(ti, ex, nw, off):
                        first, last = (ti == 0), (ti == nti - 1)
                        for cs in range(4):
                            nc.tensor.matmul(
                                oacc[cs][:, off:],
                                lhsT=kvc_sb[:, ti, cs * 128 : (cs + 1) * 128],
                                rhs=ex[:, :nw],
                                start=first,
                                stop=last,
                            )
                        nc.tensor.matmul(
                            l_ps[:, off:],
                            lhsT=onec_sb[:],
                            rhs=ex[:, :nw],
                            start=first,
                            stop=last,
                        )

                    # software-pipelined: oacc(i-1) emitted after scores(i)+
                    # exp(i) so the PE never waits on the exp latency
                    pend = None
                    for ti in range(nti):
                        tsl = slice(ti * 128, (ti + 1) * 128)
                        off = max(0, ti * 128 - qc * CH)
                        nw = CH - off  # live sq columns (diag tiles shrink)
                        sc = psB.tile([P, CH], F32, tag="mm")
                        for cs in range(4):
                            nc.tensor.matmul(
                                sc[:, :nw],
                                lhsT=kvcT_sb[:, cs, tsl],
                                rhs=qp[:, cs, off:],
                                start=(cs == 0),
                                stop=False,
                            )
                        nc.tensor.matmul(
                            sc[:, :nw],
                            lhsT=k_pe[:, tsl],
                            rhs=q_pe[:, off:],
                            start=False,
                            stop=True,
                        )
                        ex = expp.tile([P, CH], BF16, tag="exp")
                        nc.scalar.activation(ex[:, :nw], sc[:, :nw], AF.Exp)
                        if ti * 128 - qc * CH >= 0:  # diagonal: causal 0/1 mask
                            nc.vector.tensor_tensor(
                                ex[:, :nw],
                                ex[:, :nw],
                                mmst_sb[:, :nw],
                                ALU.mult,
                            )
                        if pend is not None:
                            emit_oacc(*pend)
                        pend = (ti, ex, nw, off)
                    emit_oacc(*pend)
                    # absorb the NEXT head during this head's tail (the next
                    # chunk's head 0 waits until after the output projection,
                    # which needs psB first)
                    if hh < HG - 1:
                        qp_next = absorb(qc, hh + 1)
                    # 1/l broadcast across partitions via K=1 matmul
                    rl = f32p.tile([1, CH], F32R, tag="f32")
                    with nc.allow_low_precision(reason="1/l bcast via f32r matmul"):
                        nc.vector.reciprocal(rl[:], l_ps)
                    bc_ps = psL.tile([P, CH], F32, tag="lacc")
                    nc.tensor.matmul(
                        bc_ps, lhsT=oner_sb[:], rhs=rl[:], start=True, stop=True
                    )
                    bc = f32p.tile([P, CH], F32, tag="f32")
                    nc.scalar.copy(bc[:], bc_ps)
                    # value up-projection
                    ov = [
                        ovp.tile([P, CH], BF16, tag="ov", name=f"ov{i}")
                        for i in range(4)
                    ]
                    for cs in range(4):
                        if cs % 2 == 0:
                            nc.vector.tensor_copy(ov[cs][:], oacc[cs])
                        else:
                            nc.scalar.copy(ov[cs][:], oacc[cs])
                    of_ps = psC.tile([P, CH], F32, tag="aux")
                    for cs in range(4):
                        nc.tensor.matmul(
                            of_ps,
                            lhsT=wbvT_sb[:, cs, hh, :],
                            rhs=ov[cs][:],
                            start=(cs == 0),
                            stop=(cs == 3),
                        )
                    nc.vector.tensor_tensor(ocat[:, hh, :], of_ps, bc[:], ALU.mult)
                # ---- output projection for this chunk ----
                for st2 in range(4):
                    st = qc * 4 + st2
                    stg = stgp.tile([P, DIM], F32, tag="stg")
                    for dc in range(4):
                        op = psA.tile([P, CH], F32, tag="oacc")
                        for es in range(4):
                            nc.tensor.matmul(
                                op,
                                lhsT=ocat[:, es, st2 * 128 : (st2 + 1) * 128],
                                rhs=woT_sb[:, es, dc * CH : (dc + 1) * CH],
                                start=(es == 0),
                                stop=(es == 3),
                            )
                        dsl = slice(dc * CH, (dc + 1) * CH)
                        if dc % 2 == 0:
                            nc.vector.tensor_copy(stg[:, dsl], op)
                        else:
                            nc.scalar.copy(stg[:, dsl], op)
                    nc.sync.dma_start(outp[:, st, :], stg[:])
                if qc < NCH - 1:
                    qp_next = absorb(qc + 1, 0)


# --- walrus in this container rejects >1 sem-wait per instruction; split ---
def _split_excess_waits(nc, max_waits=1):
    for f in nc.m.functions:
        for bb in f.blocks:
            if not any(
                i.sync_info is not None and len(i.sync_info.on_wait) > max_waits
                for i in bb.instructions
            ):
                continue
            new_insts = []
            for inst in bb.instructions:
                si = inst.sync_info
                if si is not None and len(si.on_wait) > max_waits:
                    waits = list(si.on_wait)
                    extra, keep = waits[:-max_waits], waits[-max_waits:]
                    for j in range(0, len(extra), max_waits):
                        nop = mybir.InstNoOp(
                            name=f"{inst.name}-wsplit-{j}", ins=[], outs=[]
                        )
                        nop.engine = inst.engine
                        nop.sync_info = mybir.SyncInfo(
                            on_wait=extra[j : j + max_waits], on_update=[]
                        )
                        new_insts.append(nop)
                    inst.sync_info = mybir.SyncInfo(
                        on_wait=keep, on_update=list(si.on_update)
                    )
                new_insts.append(inst)
            bb.instructions = new_insts


_NC = {}


def _module(reps=1):
    if reps not in _NC:
        nc = bass.Bass(
            "TRN2", target_bir_lowering=False, debug=False, num_devices=8
        )
        _emit(nc, reps=reps)
        _split_excess_waits(nc)
        _NC[reps] = nc
    return _NC[reps]


def _prep_core(core, x, wq, wkv_a, kv_norm_w, wkv_b, wo, fc, fs):
    """Build the per-core input map (numpy, host-side sharding + layouts)."""
    b, g = core // 4, core % 4
    heads = [4 * g + i for i in range(HG)]

    def bf(a):
        return np.ascontiguousarray(a.astype(NPBF16))

    m = {}
    xx = x[b]  # [S, DIM]
    m["xT"] = bf(xx.reshape(NCH, CH, NT, P).transpose(3, 0, 2, 1))

    rows_n = np.concatenate([h * QKHD + np.arange(NOPE) for h in heads])
    rows_lo = np.concatenate([h * QKHD + NOPE + 2 * np.arange(32) for h in heads])
    rows_hi = np.concatenate([h * QKHD + NOPE + 2 * np.arange(32) + 1 for h in heads])
    wqs = (wq * SCALE).astype(np.float32)
    for nm, rows in (("wqn", rows_n), ("wqlo", rows_lo), ("wqhi", rows_hi)):
        sel = wqs[rows]  # [M, DIM]
        m[nm] = bf(sel.T.reshape(NT, P, len(rows)).transpose(1, 0, 2))

    krows = np.concatenate(
        [np.arange(KLR), KLR + 2 * np.arange(32), KLR + 2 * np.arange(32) + 1]
    )
    m["wkva"] = bf(wkv_a[krows].T.reshape(NT, P, 576).transpose(1, 0, 2))

    wb = wkv_b.reshape(H, NOPE + VHD, KLR)
    wk = wb[heads, :NOPE, :] * kv_norm_w[None, None, :]  # [HG, d, c]
    m["wbk"] = bf(wk.transpose(1, 0, 2))  # [p=d, hh, c]
    wv = wb[heads, NOPE:, :] * kv_norm_w[None, None, :]  # [HG, d, c]
    m["wbvT"] = bf(wv.transpose(2, 0, 1).reshape(4, P, HG, VHD).transpose(1, 0, 2, 3))

    wo_s = wo[:, 4 * g * VHD : 4 * (g + 1) * VHD]  # [DIM, 512]
    m["woT"] = bf(wo_s.T.reshape(4, P, DIM).transpose(1, 0, 2))

    m["cosS"] = bf(np.tile(fc.T, (4, 1)))
    m["sinS"] = bf(np.tile(fs.T, (4, 1)))
    m["cosT"] = bf(fc.reshape(NT, P, 32).transpose(1, 0, 2))
    m["sinT"] = bf(fs.reshape(NT, P, 32).transpose(1, 0, 2))

    pp = np.arange(P)[:, None]
    uu = np.arange(512)[None, :]
    m["mmst"] = bf((pp <= uu).astype(np.float32))
    m["onec"] = bf(np.ones((P, 1), np.float32))
    m["oner"] = np.ones((1, P), np.float32)
    m["ident"] = bf(np.eye(P, dtype=np.float32))
    m["epsb"] = np.full((P, 1), EPS, np.float32)
    return m


def _make_in_maps(inputs):
    x = np.asarray(inputs["x"], np.float32)
    wq = np.asarray(inputs["wq"], np.float32)
    wkv_a = np.asarray(inputs["wkv_a"], np.float32)
    kv_norm_w = np.asarray(inputs["kv_norm_w"], np.float32)
    wkv_b = np.asarray(inputs["wkv_b"], np.float32)
    wo = np.asarray(inputs["wo"], np.float32)
    fc = np.asarray(inputs["freqs_cos"], np.float32)
    fs = np.asarray(inputs["freqs_sin"], np.float32)
    return [
        _prep_core(c, x, wq, wkv_a, kv_norm_w, wkv_b, wo, fc, fs) for c in range(8)
    ]


def _assemble(results):
    out = np.zeros((B, S, DIM), np.float32)
    for c in range(8):
        b = c // 4
        part = results[c]["outp"]  # [P, NT, DIM]
        out[b] += part.transpose(1, 0, 2).reshape(S, DIM)
    return out


def kernel(**inputs):
    nc = _module()
    in_maps = _make_in_maps(inputs)
    res = bass_utils.run_bass_kernel_spmd(nc, in_maps, core_ids=list(range(8)))
    return _assemble(res.results)
